# revision 11
# baseline (speedup 1.0000x reference)
"""TRN2 Bass kernel for nn_Aggregator (GNN message passing aggregator).

Strategy (8 NeuronCores, SPMD):
  - Sort edges by head (host), shard by head range: core c owns entities
    [c*12544, (c+1)*12544) and all edges whose head falls in that range.
  - Per 128-entity tile, edges are padded to CPT chunks of 128 slots.
  - Stage A (per core): per-edge gather entity_emb[tail] (indirect DMA),
    one-hot matmul machinery computes attention + unnormalized kg per tile
    in PSUM (factor-out normalization -> single pass), then
    G = (kg^2) @ (weight^2).T per tile ([ent, 32]).
  - AllGather G ([12544,32] per core -> [100352,32] everywhere). Only
    collective in the kernel (w = (|kg_h*rel| |kg_t*rel|)^2 only needs
    G[h,type]*G[t,type]).
  - Stage B: per-edge gather of combined rows [emb(128) | G(32)] built in
    DRAM, unstable scatter-softmax (exact: max w ~ 6e-4 on this data), output
    accumulated per tile via one-hot matmuls, written as the core's shard.
"""
import sys

for _p in ("/opt/trn_rl_repo", "/root/.axon_site/_ro/trn_rl_repo"):
    if _p not in sys.path:
        sys.path.insert(0, _p)

import numpy as np

import concourse.bass as bass
import concourse.bacc as bacc
import concourse.mybir as mybir
import concourse.tile as tile
from concourse.bass import IndirectOffsetOnAxis
from concourse.bass_utils import run_bass_kernel_spmd
from concourse.masks import make_identity

FP = mybir.dt.float32
I32 = mybir.dt.int32

# Problem constants (hardcoded per contest contract)
N_ENT = 100000
D = 128
H = 4
DH = 32
R = 32
NCORE = 8
TILE = 128
TPC = 98          # tiles per core
N_PER = TPC * TILE  # 12544
N_PAD = N_PER * NCORE  # 100352
CPT = 5           # chunks (of 128 edge slots) per tile; max tile load is 576
CW = 161          # combined row: 32 G | 1 ones | 128 emb


def build(ncore=NCORE, tpc=TPC, cpt=CPT, n_tab=N_ENT, with_cc=True, upto='full'):
    """Build the SPMD Bass program. Parameterized for small-scale testing."""
    n_per = tpc * TILE
    n_pad = n_per * ncore
    nch = tpc * cpt

    nc = bacc.Bacc()
    ent = nc.dram_tensor("ent", [n_pad, D], FP, kind="ExternalInput")
    myrows = nc.dram_tensor("myrows", [n_per, D], FP, kind="ExternalInput")
    tailidx = nc.dram_tensor("tailidx", [128, nch], I32, kind="ExternalInput")
    hloc = nc.dram_tensor("hloc", [128, nch], FP, kind="ExternalInput")
    typ = nc.dram_tensor("typ", [128, nch], FP, kind="ExternalInput")
    qT = nc.dram_tensor("qT", [D, D], FP, kind="ExternalInput")
    kT = nc.dram_tensor("kT", [D, D], FP, kind="ExternalInput")
    vT = nc.dram_tensor("vT", [D, D], FP, kind="ExternalInput")
    wgt = nc.dram_tensor("wgt", [R, D], FP, kind="ExternalInput")
    w2T = nc.dram_tensor("w2T", [D, R], FP, kind="ExternalInput")
    hmask = nc.dram_tensor("hmask", [D, H], FP, kind="ExternalInput")
    hmaskT = nc.dram_tensor("hmaskT", [H, D], FP, kind="ExternalInput")
    iota_in = nc.dram_tensor("iota", [128, 128], FP, kind="ExternalInput")
    out_d = nc.dram_tensor("out", [n_per, D], FP, kind="ExternalOutput")

    g_my = nc.dram_tensor("g_my", [n_per, R + 1], FP)
    if ncore > 4:
        g_full = nc.dram_tensor("g_full", [n_pad, R + 1], FP, addr_space="Shared")
    else:
        g_full = nc.dram_tensor("g_full", [n_pad, R + 1], FP)
    comb = nc.dram_tensor("comb", [n_pad, CW], FP)

    with tile.TileContext(nc) as tc:
        with (
            tc.tile_pool(name="consts", bufs=1) as consts,
            tc.tile_pool(name="asb", bufs=3) as asb,
            tc.tile_pool(name="tps", bufs=2, space="PSUM") as tps,
            tc.tile_pool(name="acc", bufs=1, space="PSUM") as accp,
            tc.tile_pool(name="tsb", bufs=2) as tsb,
        ):
            # ---------- preload constants / indices ----------
            ident = consts.tile([128, 128], FP, tag="ident")
            make_identity(nc, ident[:])
            qT_s = consts.tile([D, D], FP, tag="qT")
            kT_s = consts.tile([D, D], FP, tag="kT")
            vT_s = consts.tile([D, D], FP, tag="vT")
            wgt_s = consts.tile([R, D], FP, tag="wgt")
            w2T_s = consts.tile([D, R], FP, tag="w2T")
            hm_s = consts.tile([D, H], FP, tag="hm")
            hmT_s = consts.tile([H, D], FP, tag="hmT")
            iota_s = consts.tile([128, 128], FP, tag="iota")
            for dst, src in ((qT_s, qT), (kT_s, kT), (vT_s, vT), (wgt_s, wgt),
                             (w2T_s, w2T), (hm_s, hmask), (hmT_s, hmaskT),
                             (iota_s, iota_in)):
                nc.sync.dma_start(out=dst[:], in_=src[:])
            tidx_s = consts.tile([128, nch], I32, tag="tidx")
            hloc_s = consts.tile([128, nch], FP, tag="hloc")
            typ_s = consts.tile([128, nch], FP, tag="typ")
            nc.sync.dma_start(out=tidx_s[:], in_=tailidx[:])
            nc.sync.dma_start(out=hloc_s[:], in_=hloc[:])
            nc.sync.dma_start(out=typ_s[:], in_=typ[:])

            # combined table emb part (overlaps stage A); split: AP counts
            # are 16-bit fields in the DMA ISA
            hh = n_pad // 2
            nc.sync.dma_start(out=comb[0:hh, R + 1:CW], in_=ent[0:hh, :])
            nc.sync.dma_start(out=comb[hh:n_pad, R + 1:CW], in_=ent[hh:n_pad, :])

            # ---------- stage A ----------
            for t in range(tpc):
                erow = tsb.tile([128, D], FP, tag="erow")
                nc.sync.dma_start(out=erow[:], in_=myrows[t * 128:(t + 1) * 128, :])
                eq_ps = tps.tile([128, 256], FP, tag="eq_ps")
                nc.tensor.transpose(out=eq_ps[:, 0:128], in_=erow[:], identity=ident[:])
                E_T = tsb.tile([128, D], FP, tag="E_T")
                nc.vector.tensor_copy(out=E_T[:], in_=eq_ps[:, 0:128])
                nc.tensor.matmul(out=eq_ps[:, 128:256], lhsT=E_T[:], rhs=qT_s[:],
                                 start=True, stop=True)
                Q_s = tsb.tile([128, D], FP, tag="Q_s")
                nc.vector.tensor_copy(out=Q_s[:], in_=eq_ps[:, 128:256])

                kgu = accp.tile([128, 132], FP, tag="kgu")  # 0:128 kgu | 128:132 att_norm
                for k in range(cpt):
                    j = t * cpt + k
                    # gather entity_emb[tail]
                    Te = asb.tile([128, D], FP, tag="Te")
                    nc.gpsimd.indirect_dma_start(
                        out=Te[:], out_offset=None, in_=ent[:],
                        in_offset=IndirectOffsetOnAxis(ap=tidx_s[:, j:j + 1], axis=0),
                    )
                    oR_e = asb.tile([128, R], FP, tag="oR_e")
                    nc.vector.tensor_tensor(
                        out=oR_e[:], in0=typ_s[:, j:j + 1].to_broadcast([128, R]),
                        in1=iota_s[:, 0:R], op=mybir.AluOpType.is_equal)
                    oh_e = asb.tile([128, 128], FP, tag="oh_e")
                    nc.vector.tensor_tensor(
                        out=oh_e[:], in0=hloc_s[:, j:j + 1].to_broadcast([128, 128]),
                        in1=iota_s[:], op=mybir.AluOpType.is_equal)

                    trans = tps.tile([128, 512], FP, tag="trans")
                    nc.tensor.transpose(out=trans[0:R, 0:128], in_=oR_e[:], identity=ident[:])
                    nc.tensor.transpose(out=trans[:, 128:256], in_=Te[:], identity=ident[:])
                    nc.tensor.transpose(out=trans[:, 256:384], in_=oh_e[:], identity=ident[:])
                    oRT = asb.tile([R, 128], FP, tag="oRT")
                    nc.scalar.activation(out=oRT[:], in_=trans[0:R, 0:128],
                                         func=mybir.ActivationFunctionType.Copy)
                    T_T = asb.tile([128, 128], FP, tag="T_T")
                    nc.vector.tensor_copy(out=T_T[:], in_=trans[:, 128:256])
                    oh_ent = asb.tile([128, 128], FP, tag="oh_ent")
                    nc.vector.tensor_copy(out=oh_ent[:], in_=trans[:, 256:384])

                    mm = tps.tile([128, 512], FP, tag="mm")
                    nc.tensor.matmul(out=mm[:, 0:128], lhsT=wgt_s[:], rhs=oRT[:],
                                     start=True, stop=True)  # rel_T [D, e]
                    neigh = asb.tile([128, 128], FP, tag="neigh")
                    nc.vector.tensor_mul(out=neigh[:], in0=T_T[:], in1=mm[:, 0:128])
                    nc.tensor.matmul(out=mm[:, 128:256], lhsT=kT_s[:], rhs=neigh[:],
                                     start=True, stop=True)  # k_T [D, e]
                    nc.tensor.matmul(out=mm[:, 256:384], lhsT=neigh[:], rhs=vT_s[:],
                                     start=True, stop=True)  # v_e [e, D]
                    nc.tensor.matmul(out=mm[:, 384:512], lhsT=Q_s[:], rhs=oh_ent[:],
                                     start=True, stop=True)  # q_edge_T [D, e]
                    kTs = asb.tile([128, 128], FP, tag="kTs")
                    nc.scalar.activation(out=kTs[:], in_=mm[:, 128:256],
                                         func=mybir.ActivationFunctionType.Copy)
                    v_sb = asb.tile([128, 128], FP, tag="v_sb")
                    nc.scalar.activation(out=v_sb[:], in_=mm[:, 256:384],
                                         func=mybir.ActivationFunctionType.Copy)
                    qk = asb.tile([128, 128], FP, tag="qk")
                    nc.vector.tensor_mul(out=qk[:], in0=kTs[:], in1=mm[:, 384:512])
                    nc.tensor.matmul(out=trans[0:H, 384:512], lhsT=hm_s[:], rhs=qk[:],
                                     start=True, stop=True)  # att_T [H, e]
                    attc = asb.tile([H, 128], FP, tag="attc")
                    nc.vector.tensor_scalar_min(out=attc[:], in0=trans[0:H, 384:512],
                                                scalar1=10.0)
                    nc.vector.tensor_scalar_max(out=attc[:], in0=attc[:], scalar1=-10.0)
                    expT = asb.tile([H, 128], FP, tag="expT")
                    nc.scalar.activation(out=expT[:], in_=attc[:],
                                         func=mybir.ActivationFunctionType.Exp)
                    # expatt edge-major [e, H] via transpose (reuse trans cols 128:132)
                    nc.tensor.transpose(out=trans[:, 128:132], in_=expT[:],
                                        identity=ident[0:H, 0:H])
                    # att_exp_e [e, D] (reuse trans cols 256:384)
                    nc.tensor.matmul(out=trans[:, 256:384], lhsT=expT[:], rhs=hmT_s[:],
                                     start=True, stop=True)
                    vx = asb.tile([128, 132], FP, tag="vx")
                    nc.vector.tensor_mul(out=vx[:, 0:128], in0=v_sb[:],
                                         in1=trans[:, 256:384])
                    nc.vector.tensor_copy(out=vx[:, 128:132], in_=trans[:, 128:132])
                    nc.tensor.matmul(out=kgu[:, 0:132], lhsT=oh_e[:], rhs=vx[:],
                                     start=(k == 0), stop=(k == cpt - 1))

                # tile epilogue: kg = kgu * 1/(att_norm + 1e-8); G = kg2_T.T @ w2T
                rnorm = tsb.tile([128, H], FP, tag="rnorm")
                nc.vector.tensor_scalar_add(out=rnorm[:], in0=kgu[:, 128:132],
                                            scalar1=1e-8)
                nc.vector.reciprocal(out=rnorm[:], in_=rnorm[:])
                kg_sb = tsb.tile([128, D], FP, tag="kg_sb")
                for h in range(H):
                    nc.vector.tensor_scalar_mul(
                        out=kg_sb[:, h * DH:(h + 1) * DH],
                        in0=kgu[:, h * DH:(h + 1) * DH],
                        scalar1=rnorm[:, h:h + 1])
                gp = tps.tile([128, 256], FP, tag="eq_ps")
                nc.tensor.transpose(out=gp[:, 0:128], in_=kg_sb[:], identity=ident[:])
                kg2T = tsb.tile([128, 128], FP, tag="kg2T")
                nc.scalar.square(out=kg2T[:], in_=gp[:, 0:128])
                nc.tensor.matmul(out=gp[:, 128:128 + R], lhsT=kg2T[:], rhs=w2T_s[:],
                                 start=True, stop=True)
                g_sb = tsb.tile([128, R + 1], FP, tag="g_sb")
                nc.vector.tensor_copy(out=g_sb[:, 0:R], in_=gp[:, 128:128 + R])
                nc.vector.memset(g_sb[:, R:R + 1], 1.0)
                nc.sync.dma_start(out=g_my[t * 128:(t + 1) * 128, :], in_=g_sb[:])

            if upto == 'a':
                for t in range(tpc):
                    z_sb = tsb.tile([128, D], FP, tag="o_sb")
                    nc.vector.memset(z_sb[:], 0.0)
                    nc.sync.dma_start(out=out_d[t * 128:(t + 1) * 128, :], in_=z_sb[:])
            # ---------- AllGather G ----------
            if upto != 'a' and with_cc:
                nc.gpsimd.collective_compute(
                    "AllGather", mybir.AluOpType.bypass,
                    replica_groups=[list(range(ncore))],
                    ins=[g_my[:, :]], outs=[g_full[:, :]],
                )
                nc.sync.dma_start(out=comb[0:hh, 0:R + 1], in_=g_full[0:hh, :])
                nc.sync.dma_start(out=comb[hh:n_pad, 0:R + 1], in_=g_full[hh:n_pad, :])
            elif upto != 'a':
                nc.sync.dma_start(out=g_full[0:n_per, :], in_=g_my[:, :])
                nc.sync.dma_start(out=comb[0:n_per, 0:R + 1], in_=g_my[:, :])

            if upto == 'ag':
                for t in range(tpc):
                    z_sb = tsb.tile([128, D], FP, tag="o_sb")
                    nc.vector.memset(z_sb[:], 0.0)
                    nc.sync.dma_start(out=out_d[t * 128:(t + 1) * 128, :], in_=z_sb[:])
            # ---------- stage B ----------
            bstages = {'bg': 1, 'bt': 2, 'bm': 3, 'full': 99}
            blevel = bstages.get(upto, 0)
            for t in range(tpc if blevel else 0):
                g_tile = tsb.tile([128, R], FP, tag="g_tile")
                nc.sync.dma_start(out=g_tile[:], in_=g_my[t * 128:(t + 1) * 128, 0:R])
                gtp = tps.tile([128, 128], FP, tag="eq_ps")
                nc.tensor.transpose(out=gtp[0:R, 0:128], in_=g_tile[:], identity=ident[:])
                G_T = tsb.tile([R, 128], FP, tag="G_T")
                nc.vector.tensor_copy(out=G_T[:], in_=gtp[0:R, 0:128])

                sout = accp.tile([128, 132], FP, tag="sout")  # 0 s | 1:129 outu
                for k in range(cpt):
                    j = t * cpt + k
                    Ce = asb.tile([128, CW], FP, tag="Ce")
                    nc.gpsimd.indirect_dma_start(
                        out=Ce[:], out_offset=None, in_=comb[:],
                        in_offset=IndirectOffsetOnAxis(ap=tidx_s[:, j:j + 1], axis=0),
                    )
                    oR_e = asb.tile([128, R], FP, tag="oR_e")
                    nc.vector.tensor_tensor(
                        out=oR_e[:], in0=typ_s[:, j:j + 1].to_broadcast([128, R]),
                        in1=iota_s[:, 0:R], op=mybir.AluOpType.is_equal)
                    oh_e = asb.tile([128, 128], FP, tag="oh_e")
                    nc.vector.tensor_tensor(
                        out=oh_e[:], in0=hloc_s[:, j:j + 1].to_broadcast([128, 128]),
                        in1=iota_s[:], op=mybir.AluOpType.is_equal)
                    expw = asb.tile([128, 1], FP, tag="expw")
                    if blevel >= 2:
                        btp = tps.tile([128, 256], FP, tag="mm")
                        nc.tensor.transpose(out=btp[0:R, 0:128], in_=oR_e[:], identity=ident[:])
                        oRT = asb.tile([R, 128], FP, tag="oRT")
                        nc.scalar.activation(out=oRT[:], in_=btp[0:R, 0:128],
                                             func=mybir.ActivationFunctionType.Copy)
                        nc.tensor.matmul(out=btp[:, 128:256], lhsT=oRT[:], rhs=G_T[:],
                                         start=True, stop=True)  # M1_T [e, ent]
                        scr = asb.tile([128, 128], FP, tag="scr")
                        hr2 = asb.tile([128, 1], FP, tag="hr2")
                        nc.vector.tensor_mul(out=scr[:], in0=btp[:, 128:256], in1=oh_e[:])
                        nc.vector.tensor_reduce(out=hr2[:], in_=scr[:],
                                                axis=mybir.AxisListType.X,
                                                op=mybir.AluOpType.add)
                        scr2 = asb.tile([128, R], FP, tag="scr2")
                        tr2 = asb.tile([128, 1], FP, tag="tr2")
                        nc.vector.tensor_mul(out=scr2[:], in0=Ce[:, 0:R], in1=oR_e[:])
                        nc.vector.tensor_reduce(out=tr2[:], in_=scr2[:],
                                                axis=mybir.AxisListType.X,
                                                op=mybir.AluOpType.add)
                        nc.vector.tensor_mul(out=expw[:], in0=hr2[:], in1=tr2[:])
                        nc.scalar.activation(out=expw[:], in_=expw[:],
                                             func=mybir.ActivationFunctionType.Exp)
                    else:
                        nc.vector.memset(expw[:], 1.0)
                    mske = asb.tile([128, 128], FP, tag="mske")
                    nc.vector.tensor_scalar_mul(out=mske[:], in0=oh_e[:],
                                                scalar1=expw[:])
                    nc.tensor.matmul(out=sout[:, 0:129], lhsT=mske[:],
                                     rhs=Ce[:, R:CW],
                                     start=(k == 0), stop=(k == cpt - 1))

                rs = tsb.tile([128, 1], FP, tag="rs")
                nc.vector.tensor_scalar_add(out=rs[:], in0=sout[:, 0:1],
                                            scalar1=1e-30)
                nc.vector.reciprocal(out=rs[:], in_=rs[:])
                o_sb = tsb.tile([128, D], FP, tag="o_sb")
                nc.vector.tensor_scalar_mul(out=o_sb[:], in0=sout[:, 1:129],
                                            scalar1=rs[:])
                nc.sync.dma_start(out=out_d[t * 128:(t + 1) * 128, :], in_=o_sb[:])

    nc.finalize()
    return nc


def host_prep(entity_emb, weight, qTrans, kTrans, vTrans, edge_index, edge_type,
              ncore=NCORE, tpc=TPC, cpt=CPT, n_tab=N_ENT):
    """Sort/shard/pad edges; build all per-core input dicts."""
    n_per = tpc * TILE
    nch = tpc * cpt
    slots = cpt * 128

    head = np.asarray(edge_index[0], dtype=np.int64)
    tail = np.asarray(edge_index[1], dtype=np.int64)
    etype = np.asarray(edge_type, dtype=np.int64) - 1

    order = np.argsort(head, kind="stable")
    hs, ts, rs = head[order], tail[order], etype[order]
    tile_of = hs // TILE
    n_tiles = ncore * tpc
    counts = np.bincount(tile_of, minlength=n_tiles)
    assert counts.max() <= slots, f"tile overflow: {counts.max()} > {slots}"
    tstart = np.concatenate([[0], np.cumsum(counts)])

    tails_sl = np.zeros((ncore, tpc, slots), dtype=np.int32)
    hloc_sl = np.full((ncore, tpc, slots), 255, dtype=np.float32)
    type_sl = np.full((ncore, tpc, slots), R, dtype=np.float32)
    for g in range(n_tiles):
        c, t = g // tpc, g % tpc
        n = counts[g]
        sl = slice(tstart[g], tstart[g] + n)
        tails_sl[c, t, :n] = ts[sl]
        hloc_sl[c, t, :n] = hs[sl] - g * TILE
        type_sl[c, t, :n] = rs[sl]

    # device layout [128, nch]: slot (j, p) = tile j//cpt, chunk j%cpt, lane p
    def to_dev(a, dt):
        return np.ascontiguousarray(
            a.reshape(ncore, nch, 128).transpose(0, 2, 1)).astype(dt)

    tails_d = to_dev(tails_sl, np.int32)
    hloc_d = to_dev(hloc_sl, np.float32)
    type_d = to_dev(type_sl, np.float32)

    n_pad_rows = ncore * n_per
    ent_raw = np.asarray(entity_emb, dtype=np.float32)
    ent = np.zeros((n_pad_rows, D), np.float32)
    ent[:ent_raw.shape[0]] = ent_raw
    wgt = np.asarray(weight, dtype=np.float32)
    w2T = np.ascontiguousarray((wgt ** 2).T)
    hmask = np.zeros((D, H), np.float32)
    for h in range(H):
        hmask[h * DH:(h + 1) * DH, h] = 1.0
    hmaskT = np.ascontiguousarray(hmask.T)
    iota = np.tile(np.arange(128, dtype=np.float32), (128, 1))

    shared = {
        "ent": ent,
        "qT": np.asarray(qTrans, dtype=np.float32),
        "kT": np.asarray(kTrans, dtype=np.float32),
        "vT": np.asarray(vTrans, dtype=np.float32),
        "wgt": wgt, "w2T": w2T, "hmask": hmask, "hmaskT": hmaskT, "iota": iota,
    }
    in_maps = []
    for c in range(ncore):
        myrows = np.zeros((n_per, D), np.float32)
        lo = c * n_per
        hi = min(n_tab, lo + n_per)
        if hi > lo:
            myrows[:hi - lo] = ent[lo:hi]
        in_maps.append(dict(shared, myrows=myrows, tailidx=tails_d[c],
                            hloc=hloc_d[c], typ=type_d[c]))
    return in_maps


_NC_CACHE = {}


def kernel(entity_emb, user_emb, interact_mat, weight, qTrans, kTrans, vTrans,
           edge_index, edge_type, layer=0):
    key = "full"
    if key not in _NC_CACHE:
        _NC_CACHE[key] = build()
    nc = _NC_CACHE[key]
    in_maps = host_prep(entity_emb, weight, qTrans, kTrans, vTrans,
                        edge_index, edge_type)
    res = run_bass_kernel_spmd(nc, in_maps, list(range(NCORE)))
    out = np.concatenate([res.results[c]["out"] for c in range(NCORE)], axis=0)
    return np.ascontiguousarray(out[:N_ENT]).astype(np.float32)


# revision 15
# speedup vs baseline: 2.1706x; 2.1706x over previous
"""TRN2 Bass kernel for nn_Aggregator (GNN message passing aggregator).

Strategy (8 NeuronCores, SPMD):
  - Sort edges by head (host), shard by head range: core c owns entities
    [c*12544, (c+1)*12544) and all edges whose head falls in that range.
  - Per 128-entity tile, edges are padded to CPT chunks of 128 slots;
    compute is batched per tile (S = CPT*128 edge slots) in bf16 with
    f32 PSUM accumulation.
  - Stage A: per-edge gather of entity_emb[tail] (indirect DMA, 128
    rows/call, Q7 descriptor-gen bound), one-hot matmul machinery computes
    attention + unnormalized kg per tile in PSUM (factor-out normalization
    -> single pass), then G = (kg^2) @ (weight^2).T per tile ([ent, 32]).
  - AllGather G (the only collective: the edge weight
    w = (|kg_h*rel| |kg_t*rel|)^2 equals G[h,type]*G[t,type]).
  - Stage B: per-edge gather of combined bf16 rows [G(32) | 1 | emb(128)]
    built in DRAM; unstable scatter-softmax (exact here: max w ~ 6e-4);
    one fused matmul accumulates [s | out_unnorm] per tile; output = shard.
"""
import sys

for _p in ("/opt/trn_rl_repo", "/root/.axon_site/_ro/trn_rl_repo"):
    if _p not in sys.path:
        sys.path.insert(0, _p)

import numpy as np
import ml_dtypes

import concourse.bass as bass
import concourse.bacc as bacc
import concourse.mybir as mybir
import concourse.tile as tile
from concourse.bass import IndirectOffsetOnAxis
from concourse.bass_utils import run_bass_kernel_spmd
from concourse.masks import make_identity

FP = mybir.dt.float32
BF = mybir.dt.bfloat16
I32 = mybir.dt.int32

# Problem constants
N_ENT = 100000
D = 128
H = 4
DH = 32
R = 32
NCORE = 8
TILE = 128
TPC = 98            # tiles per core
N_PER = TPC * TILE  # 12544
N_PAD = N_PER * NCORE  # 100352
CPT = 5             # chunks (128 edge slots) per tile; max tile load is 576
CW = 161            # combined row: 32 G | 1 ones | 128 emb


def _bcast(src_ap, parts):
    """Partition-broadcast a [1, S] DRAM AP to [parts, S] for DMA."""
    return bass.AP(tensor=src_ap.tensor, offset=src_ap.offset,
                   ap=[[0, parts]] + [list(p) for p in src_ap.ap[1:]])


def build(ncore=NCORE, tpc=TPC, cpt=CPT, n_tab=N_ENT, with_cc=True):
    n_per = tpc * TILE
    n_pad = n_per * ncore
    nch = tpc * cpt
    S = cpt * 128

    nc = bacc.Bacc()
    ent = nc.dram_tensor("ent", [n_pad, D], BF, kind="ExternalInput")
    myrowsT = nc.dram_tensor("myrowsT", [n_per, D], BF, kind="ExternalInput")
    tailidx = nc.dram_tensor("tailidx", [128, nch], I32, kind="ExternalInput")
    hloc = nc.dram_tensor("hloc", [128, nch], FP, kind="ExternalInput")
    typ = nc.dram_tensor("typ", [128, nch], FP, kind="ExternalInput")
    hrow = nc.dram_tensor("hrow", [tpc, S], FP, kind="ExternalInput")
    trow = nc.dram_tensor("trow", [tpc, S], FP, kind="ExternalInput")
    qT = nc.dram_tensor("qT", [D, D], BF, kind="ExternalInput")
    kT = nc.dram_tensor("kT", [D, D], BF, kind="ExternalInput")
    vT = nc.dram_tensor("vT", [D, D], BF, kind="ExternalInput")
    wgt = nc.dram_tensor("wgt", [R, D], BF, kind="ExternalInput")
    w2T = nc.dram_tensor("w2T", [D, R], BF, kind="ExternalInput")
    hmask = nc.dram_tensor("hmask", [D, H], BF, kind="ExternalInput")
    hmaskT = nc.dram_tensor("hmaskT", [H, D], BF, kind="ExternalInput")
    iota_in = nc.dram_tensor("iota", [128, 128], FP, kind="ExternalInput")
    iotac_in = nc.dram_tensor("iotac", [128, S], FP, kind="ExternalInput")
    out_d = nc.dram_tensor("out", [n_per, D], FP, kind="ExternalOutput")

    g_my = nc.dram_tensor("g_my", [n_per, R + 1], BF)
    if ncore > 4:
        g_full = nc.dram_tensor("g_full", [n_pad, R + 1], BF, addr_space="Shared")
    else:
        g_full = nc.dram_tensor("g_full", [n_pad, R + 1], BF)
    comb = nc.dram_tensor("comb", [n_pad, CW], BF)

    def mmN(out_fn, lhsT, rhs_fn, n_total, **kw):
        for off in range(0, n_total, 512):
            n = min(512, n_total - off)
            nc.tensor.matmul(out=out_fn(off, n), lhsT=lhsT, rhs=rhs_fn(off, n),
                             start=True, stop=True, **kw)

    with tile.TileContext(nc) as tc:
        with (
            tc.tile_pool(name="consts", bufs=1) as consts,
            tc.tile_pool(name="asb", bufs=2) as asb,
            tc.tile_pool(name="psA", bufs=2, space="PSUM") as psA,
            tc.tile_pool(name="psB", bufs=2, space="PSUM") as psB,
            tc.tile_pool(name="acc", bufs=1, space="PSUM") as accp,
            tc.tile_pool(name="tsb", bufs=2) as tsb,
        ):
            # ---------- constants / indices ----------
            ident = consts.tile([128, 128], BF, tag="ident")
            make_identity(nc, ident[:])
            qT_s = consts.tile([D, D], BF, tag="qT")
            kT_s = consts.tile([D, D], BF, tag="kT")
            vT_s = consts.tile([D, D], BF, tag="vT")
            wgt_s = consts.tile([R, D], BF, tag="wgt")
            w2T_s = consts.tile([D, R], BF, tag="w2T")
            hm_s = consts.tile([D, H], BF, tag="hm")
            hmT_s = consts.tile([H, D], BF, tag="hmT")
            iota_s = consts.tile([128, 128], FP, tag="iota")
            iotac_s = consts.tile([128, S], FP, tag="iotac")
            for dst, src in ((qT_s, qT), (kT_s, kT), (vT_s, vT), (wgt_s, wgt),
                             (w2T_s, w2T), (hm_s, hmask), (hmT_s, hmaskT),
                             (iota_s, iota_in), (iotac_s, iotac_in)):
                nc.sync.dma_start(out=dst[:], in_=src[:])
            tidx_s = consts.tile([128, nch], I32, tag="tidx")
            hloc_s = consts.tile([128, nch], FP, tag="hloc")
            typ_s = consts.tile([128, nch], FP, tag="typ")
            nc.sync.dma_start(out=tidx_s[:], in_=tailidx[:])
            nc.sync.dma_start(out=hloc_s[:], in_=hloc[:])
            nc.sync.dma_start(out=typ_s[:], in_=typ[:])

            # combined-table emb part (overlaps stage A); 16-bit AP counts
            hh = n_pad // 2
            nc.sync.dma_start(out=comb[0:hh, R + 1:CW], in_=ent[0:hh, :])
            nc.sync.dma_start(out=comb[hh:n_pad, R + 1:CW], in_=ent[hh:n_pad, :])

            # ---------- stage A ----------
            for t in range(tpc):
                E_T = tsb.tile([128, D], BF, tag="E_T")
                nc.sync.dma_start(out=E_T[:], in_=myrowsT[t * 128:(t + 1) * 128, :])
                q_ps = psB.tile([128, 512], FP, tag="B")
                nc.tensor.matmul(out=q_ps[:, 0:128], lhsT=E_T[:], rhs=qT_s[:],
                                 start=True, stop=True)
                Q_s = tsb.tile([128, D], BF, tag="Q_s")
                nc.vector.tensor_copy(out=Q_s[:], in_=q_ps[:, 0:128])

                # broadcast head/type rows for this tile
                hbc = tsb.tile([128, S], FP, tag="hbc")
                nc.sync.dma_start(out=hbc[:], in_=_bcast(hrow[t:t + 1, :], 128))
                tbc = tsb.tile([R, S], FP, tag="tbc")
                nc.sync.dma_start(out=tbc[:], in_=_bcast(trow[t:t + 1, :], R))

                # gathered tails for the whole tile
                Te = asb.tile([128, cpt, D], BF, tag="Te")
                for k in range(cpt):
                    nc.gpsimd.indirect_dma_start(
                        out=Te[:, k, :], out_offset=None, in_=ent[:],
                        in_offset=IndirectOffsetOnAxis(
                            ap=tidx_s[:, t * cpt + k:t * cpt + k + 1], axis=0),
                    )
                # one-hots: edge-major oh_e/oR_e (columns), ent-major oh_ent,
                # feature-major oRT (rows, via broadcast + iota compare)
                oh_e = asb.tile([128, cpt, 128], BF, tag="oh_e")
                oR_e = asb.tile([128, cpt, R], BF, tag="oR_e")
                for k in range(cpt):
                    j = t * cpt + k
                    nc.vector.tensor_tensor(
                        out=oh_e[:, k, :],
                        in0=hloc_s[:, j:j + 1].to_broadcast([128, 128]),
                        in1=iota_s[:], op=mybir.AluOpType.is_equal)
                    nc.vector.tensor_tensor(
                        out=oR_e[:, k, :],
                        in0=typ_s[:, j:j + 1].to_broadcast([128, R]),
                        in1=iota_s[:, 0:R], op=mybir.AluOpType.is_equal)
                oh_ent = asb.tile([128, S], BF, tag="oh_ent")
                nc.vector.tensor_tensor(out=oh_ent[:], in0=hbc[:], in1=iotac_s[:],
                                        op=mybir.AluOpType.is_equal)
                oRT = asb.tile([R, S], BF, tag="oRT")
                nc.vector.tensor_tensor(out=oRT[:], in0=tbc[:],
                                        in1=iotac_s[0:R, :],
                                        op=mybir.AluOpType.is_equal)

                # T_T via PE transposes
                tt_ps = psA.tile([128, S], BF, tag="A")
                for k in range(cpt):
                    nc.tensor.transpose(out=tt_ps[:, k * 128:(k + 1) * 128],
                                        in_=Te[:, k, :], identity=ident[:])
                T_T = asb.tile([128, S], BF, tag="T_T")
                nc.vector.tensor_copy(out=T_T[:], in_=tt_ps[:])

                # rel_T = wgt.T @ oRT; neigh_T = T_T * rel_T
                rel_ps = psA.tile([128, S], FP, tag="A")
                mmN(lambda o, n: rel_ps[:, o:o + n], wgt_s[:],
                    lambda o, n: oRT[:, o:o + n], S)
                neigh = asb.tile([128, S], BF, tag="neigh")
                nc.vector.tensor_mul(out=neigh[:], in0=T_T[:], in1=rel_ps[:])
                # k_T = kT.T @ neigh
                k_ps = psA.tile([128, S], FP, tag="A")
                mmN(lambda o, n: k_ps[:, o:o + n], kT_s[:],
                    lambda o, n: neigh[:, o:o + n], S)
                kTs = asb.tile([128, S], BF, tag="kTs")
                nc.scalar.activation(out=kTs[:], in_=k_ps[:],
                                     func=mybir.ActivationFunctionType.Copy)
                # q_edge_T = Q.T @ oh_ent
                qe_ps = psA.tile([128, S], FP, tag="A")
                mmN(lambda o, n: qe_ps[:, o:o + n], Q_s[:],
                    lambda o, n: oh_ent[:, o:o + n], S)
                qk = asb.tile([128, S], BF, tag="qk")
                nc.vector.tensor_mul(out=qk[:], in0=kTs[:], in1=qe_ps[:])
                # att = hmask.T @ qk  [H, S] (reuse qe_ps rows 0:H after qk read)
                mmN(lambda o, n: qe_ps[0:H, o:o + n], hm_s[:],
                    lambda o, n: qk[:, o:o + n], S)
                attc = asb.tile([H, S], FP, tag="attc")
                nc.vector.tensor_scalar_min(out=attc[:], in0=qe_ps[0:H, 0:S],
                                            scalar1=10.0)
                nc.vector.tensor_scalar_max(out=attc[:], in0=attc[:], scalar1=-10.0)
                expT = asb.tile([H, S], BF, tag="expT")
                nc.scalar.activation(out=expT[:], in_=attc[:],
                                     func=mybir.ActivationFunctionType.Exp)
                # v_e, att_exp_e, expatt_e per chunk
                v_ps = psA.tile([128, S], FP, tag="A")
                ae_ps = psA.tile([128, S], FP, tag="A")
                eeB = psB.tile([128, 512], BF, tag="B")
                ee_ps = eeB[:, 0:cpt * 4]
                for k in range(cpt):
                    sl = slice(k * 128, (k + 1) * 128)
                    nc.tensor.matmul(out=v_ps[:, sl], lhsT=neigh[:, sl],
                                     rhs=vT_s[:], start=True, stop=True)
                    nc.tensor.matmul(out=ae_ps[:, sl], lhsT=expT[:, sl],
                                     rhs=hmT_s[:], start=True, stop=True)
                    nc.tensor.transpose(out=ee_ps[:, k * 4:(k + 1) * 4] if False else eeB[:, k * 4:(k + 1) * 4],
                                        in_=expT[:, sl], identity=ident[0:H, 0:H])
                v_sb = asb.tile([128, S], BF, tag="v_sb")
                nc.vector.tensor_copy(out=v_sb[:], in_=v_ps[:])
                vx = asb.tile([128, cpt, 132], BF, tag="vx")
                nc.vector.tensor_mul(
                    out=vx[:, :, 0:128],
                    in0=v_sb[:].rearrange("p (c e) -> p c e", c=cpt),
                    in1=ae_ps[:].rearrange("p (c e) -> p c e", c=cpt))
                nc.vector.tensor_copy(
                    out=vx[:, :, 128:132],
                    in_=ee_ps.rearrange("p (c e) -> p c e", c=cpt))
                kgu = accp.tile([128, 132], FP, tag="kgu")
                for k in range(cpt):
                    nc.tensor.matmul(out=kgu[:, 0:132], lhsT=oh_e[:, k, :],
                                     rhs=vx[:, k, :],
                                     start=(k == 0), stop=(k == cpt - 1))

                # tile epilogue: kg, G
                rnorm = tsb.tile([128, H], FP, tag="rnorm")
                nc.vector.tensor_scalar_add(out=rnorm[:], in0=kgu[:, 128:132],
                                            scalar1=1e-8)
                nc.vector.reciprocal(out=rnorm[:], in_=rnorm[:])
                kg_sb = tsb.tile([128, D], BF, tag="kg_sb")
                for h in range(H):
                    nc.vector.tensor_scalar_mul(
                        out=kg_sb[:, h * DH:(h + 1) * DH],
                        in0=kgu[:, h * DH:(h + 1) * DH],
                        scalar1=rnorm[:, h:h + 1])
                gp = psB.tile([128, 512], BF, tag="B")
                nc.tensor.transpose(out=gp[:, 0:128], in_=kg_sb[:], identity=ident[:])
                kg2T = tsb.tile([128, 128], BF, tag="kg2T")
                nc.scalar.square(out=kg2T[:], in_=gp[:, 0:128])
                gf = psB.tile([128, 512], FP, tag="B")
                nc.tensor.matmul(out=gf[:, 0:R], lhsT=kg2T[:], rhs=w2T_s[:],
                                 start=True, stop=True)
                g_sb = tsb.tile([128, R + 1], BF, tag="g_sb")
                nc.vector.tensor_copy(out=g_sb[:, 0:R], in_=gf[:, 0:R])
                nc.vector.memset(g_sb[:, R:R + 1], 1.0)
                nc.sync.dma_start(out=g_my[t * 128:(t + 1) * 128, :], in_=g_sb[:])

            # ---------- AllGather G ----------
            if with_cc:
                nc.gpsimd.collective_compute(
                    "AllGather", mybir.AluOpType.bypass,
                    replica_groups=[list(range(ncore))],
                    ins=[g_my[:, :]], outs=[g_full[:, :]],
                )
                nc.sync.dma_start(out=comb[0:hh, 0:R + 1], in_=g_full[0:hh, :])
                nc.sync.dma_start(out=comb[hh:n_pad, 0:R + 1], in_=g_full[hh:n_pad, :])
            else:
                nc.sync.dma_start(out=g_full[0:n_per, :], in_=g_my[:, :])
                nc.sync.dma_start(out=comb[0:n_per, 0:R + 1], in_=g_my[:, :])

            # ---------- stage B ----------
            for t in range(tpc):
                g_tile = tsb.tile([128, R], BF, tag="g_tile")
                nc.sync.dma_start(out=g_tile[:], in_=g_my[t * 128:(t + 1) * 128, 0:R])
                gt_ps = psB.tile([128, 512], BF, tag="B")
                nc.tensor.transpose(out=gt_ps[0:R, 0:128], in_=g_tile[:],
                                    identity=ident[:])
                G_T = tsb.tile([R, 128], BF, tag="G_T")
                nc.vector.tensor_copy(out=G_T[:], in_=gt_ps[0:R, 0:128])
                tbc = tsb.tile([R, S], FP, tag="tbc")
                nc.sync.dma_start(out=tbc[:], in_=_bcast(trow[t:t + 1, :], R))

                Ce = asb.tile([128, cpt, CW], BF, tag="Ce")
                for k in range(cpt):
                    nc.gpsimd.indirect_dma_start(
                        out=Ce[:, k, :], out_offset=None, in_=comb[:],
                        in_offset=IndirectOffsetOnAxis(
                            ap=tidx_s[:, t * cpt + k:t * cpt + k + 1], axis=0),
                    )
                oh_e = asb.tile([128, cpt, 128], BF, tag="oh_e")
                oR_e = asb.tile([128, cpt, R], BF, tag="oR_e")
                for k in range(cpt):
                    j = t * cpt + k
                    nc.vector.tensor_tensor(
                        out=oh_e[:, k, :],
                        in0=hloc_s[:, j:j + 1].to_broadcast([128, 128]),
                        in1=iota_s[:], op=mybir.AluOpType.is_equal)
                    nc.vector.tensor_tensor(
                        out=oR_e[:, k, :],
                        in0=typ_s[:, j:j + 1].to_broadcast([128, R]),
                        in1=iota_s[:, 0:R], op=mybir.AluOpType.is_equal)
                oRT = asb.tile([R, S], BF, tag="oRT")
                nc.vector.tensor_tensor(out=oRT[:], in0=tbc[:],
                                        in1=iotac_s[0:R, :],
                                        op=mybir.AluOpType.is_equal)
                # M1_T [e, ent] per chunk; hr2/tr2 via mult+reduce
                m1_ps = psA.tile([128, S], FP, tag="A")
                for k in range(cpt):
                    nc.tensor.matmul(out=m1_ps[:, k * 128:(k + 1) * 128],
                                     lhsT=oRT[:, k * 128:(k + 1) * 128],
                                     rhs=G_T[:], start=True, stop=True)
                scr = asb.tile([128, S], BF, tag="scr")
                nc.vector.tensor_mul(
                    out=scr[:],
                    in0=m1_ps[:],
                    in1=oh_e[:].rearrange("p c e -> p (c e)"))
                hr2 = asb.tile([128, cpt], FP, tag="hr2")
                nc.vector.tensor_reduce(
                    out=hr2[:], in_=scr[:].rearrange("p (c e) -> p c e", c=cpt),
                    axis=mybir.AxisListType.X, op=mybir.AluOpType.add)
                scr2 = asb.tile([128, cpt * R], FP, tag="scr2")
                nc.vector.tensor_mul(
                    out=scr2[:].rearrange("p (c r) -> p c r", c=cpt),
                    in0=Ce[:, :, 0:R],
                    in1=oR_e[:])
                tr2 = asb.tile([128, cpt], FP, tag="tr2")
                nc.vector.tensor_reduce(
                    out=tr2[:], in_=scr2[:].rearrange("p (c r) -> p c r", c=cpt),
                    axis=mybir.AxisListType.X, op=mybir.AluOpType.add)
                expw = asb.tile([128, cpt], FP, tag="expw")
                nc.vector.tensor_mul(out=expw[:], in0=hr2[:], in1=tr2[:])
                nc.scalar.activation(out=expw[:], in_=expw[:],
                                     func=mybir.ActivationFunctionType.Exp)
                mske = asb.tile([128, cpt, 128], BF, tag="mske")
                sout = accp.tile([128, 132], FP, tag="kgu")
                for k in range(cpt):
                    nc.vector.tensor_scalar_mul(out=mske[:, k, :],
                                                in0=oh_e[:, k, :],
                                                scalar1=expw[:, k:k + 1])
                    nc.tensor.matmul(out=sout[:, 0:129], lhsT=mske[:, k, :],
                                     rhs=Ce[:, k, R:CW],
                                     start=(k == 0), stop=(k == cpt - 1))

                rs = tsb.tile([128, 1], FP, tag="rs")
                nc.vector.tensor_scalar_add(out=rs[:], in0=sout[:, 0:1],
                                            scalar1=1e-30)
                nc.vector.reciprocal(out=rs[:], in_=rs[:])
                o_sb = tsb.tile([128, D], FP, tag="o_sb")
                nc.vector.tensor_scalar_mul(out=o_sb[:], in0=sout[:, 1:129],
                                            scalar1=rs[:])
                nc.sync.dma_start(out=out_d[t * 128:(t + 1) * 128, :], in_=o_sb[:])

    nc.finalize()
    return nc


def host_prep(entity_emb, weight, qTrans, kTrans, vTrans, edge_index, edge_type,
              ncore=NCORE, tpc=TPC, cpt=CPT, n_tab=N_ENT):
    """Sort/shard/pad edges; build all per-core input dicts."""
    n_per = tpc * TILE
    nch = tpc * cpt
    slots = cpt * 128

    head = np.asarray(edge_index[0], dtype=np.int64)
    tail = np.asarray(edge_index[1], dtype=np.int64)
    etype = np.asarray(edge_type, dtype=np.int64) - 1

    order = np.argsort(head, kind="stable")
    hs, ts, rs = head[order], tail[order], etype[order]
    tile_of = hs // TILE
    n_tiles = ncore * tpc
    counts = np.bincount(tile_of, minlength=n_tiles)
    assert counts.max() <= slots, f"tile overflow: {counts.max()} > {slots}"
    tstart = np.concatenate([[0], np.cumsum(counts)])

    tails_sl = np.zeros((ncore, tpc, slots), dtype=np.int32)
    hloc_sl = np.full((ncore, tpc, slots), 255, dtype=np.float32)
    type_sl = np.full((ncore, tpc, slots), R, dtype=np.float32)
    for g in range(n_tiles):
        c, t = g // tpc, g % tpc
        n = counts[g]
        sl = slice(tstart[g], tstart[g] + n)
        tails_sl[c, t, :n] = ts[sl]
        hloc_sl[c, t, :n] = hs[sl] - g * TILE
        type_sl[c, t, :n] = rs[sl]

    def to_dev(a, dt):
        return np.ascontiguousarray(
            a.reshape(ncore, nch, 128).transpose(0, 2, 1)).astype(dt)

    tails_d = to_dev(tails_sl, np.int32)
    hloc_d = to_dev(hloc_sl, np.float32)
    type_d = to_dev(type_sl, np.float32)

    n_pad_rows = ncore * n_per
    ent_raw = np.asarray(entity_emb, dtype=np.float32)
    ent = np.zeros((n_pad_rows, D), np.float32)
    ent[:ent_raw.shape[0]] = ent_raw
    ent_bf = ent.astype(ml_dtypes.bfloat16)

    wgt = np.asarray(weight, dtype=np.float32)
    w2T = np.ascontiguousarray((wgt ** 2).T)
    hmask = np.zeros((D, H), np.float32)
    for h in range(H):
        hmask[h * DH:(h + 1) * DH, h] = 1.0
    hmaskT = np.ascontiguousarray(hmask.T)
    iota = np.tile(np.arange(128, dtype=np.float32), (128, 1))
    iotac = np.tile(np.arange(128, dtype=np.float32)[:, None], (1, slots))

    shared = {
        "ent": ent_bf,
        "qT": np.asarray(qTrans, np.float32).astype(ml_dtypes.bfloat16),
        "kT": np.asarray(kTrans, np.float32).astype(ml_dtypes.bfloat16),
        "vT": np.asarray(vTrans, np.float32).astype(ml_dtypes.bfloat16),
        "wgt": wgt.astype(ml_dtypes.bfloat16),
        "w2T": w2T.astype(ml_dtypes.bfloat16),
        "hmask": hmask.astype(ml_dtypes.bfloat16),
        "hmaskT": hmaskT.astype(ml_dtypes.bfloat16),
        "iota": iota, "iotac": iotac,
    }
    in_maps = []
    for c in range(ncore):
        rows = ent[c * n_per:(c + 1) * n_per]
        myT = rows.reshape(tpc, TILE, D).transpose(0, 2, 1).reshape(n_per, D)
        in_maps.append(dict(
            shared,
            myrowsT=np.ascontiguousarray(myT).astype(ml_dtypes.bfloat16),
            tailidx=tails_d[c],
            hloc=hloc_d[c], typ=type_d[c],
            hrow=np.ascontiguousarray(hloc_sl[c]).astype(np.float32),
            trow=np.ascontiguousarray(type_sl[c]).astype(np.float32),
        ))
    return in_maps


_NC_CACHE = {}


def kernel(entity_emb, user_emb, interact_mat, weight, qTrans, kTrans, vTrans,
           edge_index, edge_type, layer=0):
    key = "full"
    if key not in _NC_CACHE:
        _NC_CACHE[key] = build()
    nc = _NC_CACHE[key]
    in_maps = host_prep(entity_emb, weight, qTrans, kTrans, vTrans,
                        edge_index, edge_type)
    res = run_bass_kernel_spmd(nc, in_maps, list(range(NCORE)))
    out = np.concatenate([res.results[c]["out"] for c in range(NCORE)], axis=0)
    return np.ascontiguousarray(out[:N_ENT]).astype(np.float32)
